# revision 47
# baseline (speedup 1.0000x reference)
"""Trainium2 Bass kernel for grouped per-channel linears (nn_GroupedLinearsAdvanced).

Math: out[b, o, d] = sum_i x[b, i, d] * W[d, i, o] + bias[d, o]
with x: [16, 128, 4096] f32, W: [4096, 128, 128] f32, bias: [4096, 128] f32,
out: [16, 128, 4096] f32.

Sharding: channel dim D=4096 split into 8 contiguous slabs of 512 channels,
one per NeuronCore; x slices replicated per-slab, no cross-device reduction.

Per-core dataflow (DMA-bound; ~20 MB of HBM traffic per core at ~26 GB/s
per SDMA engine x 16 engines):
  - host pre-permutes inputs so every DMA moves long contiguous
    per-partition runs; x and W are cast to bf16 on host (halves W traffic
    vs fp32; rel-err ~3e-3 against the fp32 reference),
  - x slab resident in SBUF: layout [i, dl*16+b]; its two 1 MB chunks go
    one per HWDGE ring so both rings carry equal bytes (9 MB each),
  - the whole 16 MB W slab is prefetched through SBUF in 8 x 2 MB tiles,
    all dma_starts issued up front, tiles alternating between the sync and
    scalar HWDGE rings (SDMA engines alternate descriptors between rings;
    a single ring cannot sustain line rate),
  - per channel: one matmul  PS[o, b] = W_d.T @ x_d.T  (lhsT = W_d),
    32 channels accumulate side-by-side into one 512-f32 PSUM bank,
  - bias (when nonzero) seeds each PSUM bank via a bf16 one-hot expansion
    matmul: PS[o, j*16+b] = bias[g*32+j, o] = (BN_g).T @ E; skipped when
    the supplied bias is identically zero,
  - DVE evacuates each bank to SBUF casting f32 -> bf16 (halves out DMA),
  - outs stream to HBM via the GPSIMD SWDGE queue so they never queue in
    a HWDGE ring FIFO behind undrained W packets; host casts back to f32.

MM_DTYPE picks the tensor-engine path for x/W:
  "f32"  — exact fp32 (hardware runs 2 half-speed passes per matmul),
  "f32r" — same fp32 bytes, single-pass reduced-precision PE mode,
  "bf16" — host-side cast, halves DMA traffic, single-pass matmuls + FWL,
  "mix3" — bf16 hi+lo split, fp32-class accuracy at fp32 bytes (slow path).
"""

import ml_dtypes
import numpy as np

from concourse import bacc, mybir, tile
from concourse.bass_utils import run_bass_kernel_spmd

B = 16           # batch
IN_D = 128       # contraction dim (SBUF partitions)
OUT_D = 128      # per-channel output dim
D_TOTAL = 4096   # channels
NCORES = 8
D_C = D_TOTAL // NCORES      # 512 channels per core
BANK_CH = 32                 # channels per PSUM bank (32*16 = 512 fp32 = 1 bank)
N_BANKS = D_C // BANK_CH     # 16

X_COLS = D_C * B                 # 8192
BN_COLS = N_BANKS * OUT_D        # 2048
EH_COLS = BANK_CH * B            # 512
CB_COLS = BN_COLS + EH_COLS      # bias + one-hot constant tensor

F32 = mybir.dt.float32
BF16 = mybir.dt.bfloat16

MM_DTYPE = "f8w"

F8E3 = mybir.dt.float8e3

# (w_dtype_mybir, w_dtype_np, x_dtype_mybir, x_dtype_np)
# f8w: W quantized to fp8-e3m4 (TRN FP8_EXP3, 4 mantissa bits) with a
# global power-of-2 scale (W*128, absmax 6.9 << 15.5 so no clipping);
# x carries the inverse scale (x/128, exact in bf16). Halves W HBM
# traffic vs bf16; numpy-simulated rel-err 1.22e-2 vs the fp32
# reference (threshold 2e-2). The PE allows mixed fp8-weight x
# bf16-moving matmuls (only fp32 requires matching operand dtypes).
W_SCALE = 128.0
_DT = {
    "f32": (F32, np.float32, F32, np.float32),
    "f32r": (mybir.dt.float32r, np.float32, mybir.dt.float32r, np.float32),
    "bf16": (BF16, ml_dtypes.bfloat16, BF16, ml_dtypes.bfloat16),
    "f8w": (F8E3, ml_dtypes.float8_e3m4, BF16, ml_dtypes.bfloat16),
    # mix3: W and x split into bf16 hi+lo parts; 3 single-pass matmuls
    # per channel (hi*hi + lo*hi + hi*lo) recover ~1e-5 accuracy while
    # keeping bf16 tensor-engine throughput. Same HBM bytes as fp32.
    "mix3": (BF16, ml_dtypes.bfloat16, BF16, ml_dtypes.bfloat16),
}

# --- stream layout shared by host packing and kernel build ---
# W tiles alternate rings (even index -> sync). x is packed per-ring
# contiguous (all sync tiles' channels first, then scalar's) so each
# ring's x moves as a few large-row transfers interleaved with its W
# tiles instead of one 2 KB-row sliver per tile.
W_SIZES = [32, 32, 96, 96, 112, 96, 16, 32]
assert sum(W_SIZES) == D_C
# Per-ring x piece boundaries, counted in that ring's tile list: piece 0
# covers tile 0, piece 1 tiles 1-2, piece 2 the rest. Emitted before the
# first W tile each covers.
X_PIECE_TILES = [(0, 1), (1, 99)]


def _stream_layout():
    ring_tiles = [[], []]  # tile indices per ring
    starts = []
    c0 = 0
    for i, tch in enumerate(W_SIZES):
        ring_tiles[i % 2].append(i)
        starts.append(c0)
        c0 += tch
    perm = []  # permuted channel order: sync rings' channels, then scalar
    for r in (0, 1):
        for i in ring_tiles[r]:
            perm.extend(range(starts[i], starts[i] + W_SIZES[i]))
    pos = [0] * D_C  # channel -> position in permuted x
    for p, ch in enumerate(perm):
        pos[ch] = p
    return ring_tiles, starts, perm, pos


RING_TILES, W_STARTS, X_PERM, X_POS = _stream_layout()

_cached = {}


def _build(mode, has_bias):
    w_dt, _, x_dt, _ = _DT[mode]
    nparts = 2 if mode == "mix3" else 1  # hi/lo operand copies
    out_dt = BF16 if mode in ("bf16", "f8w") else F32
    nc = bacc.Bacc()
    xc = nc.dram_tensor("xc", [IN_D, nparts * X_COLS], x_dt, kind="ExternalInput")
    wr = nc.dram_tensor(
        "wr", [IN_D, nparts * D_C * OUT_D], w_dt, kind="ExternalInput"
    )
    if has_bias:
        cb = nc.dram_tensor("cb", [BANK_CH, CB_COLS], BF16, kind="ExternalInput")
    outr = nc.dram_tensor("outr", [OUT_D, D_C * B], out_dt, kind="ExternalOutput")

    # Streaming-path (non-prefetch) tile size; the prefetch path below uses
    # `sizes` instead.
    tile_ch = 32 // nparts
    n_tiles = D_C // tile_ch
    banks_per_tile = max(1, tile_ch // BANK_CH)
    wcols_per_ch = nparts * OUT_D
    # Prefetch the whole W slab when it fits in SBUF (bf16: 8 tiles x
    # 16 KB/partition = 128 KB). All W dma_starts issue up front, so a
    # later out-DMA's semaphore wait on the sync ring can never delay a
    # W transfer (HWDGE rings are FIFO per issuing engine).
    w_elt = {F8E3: 1, BF16: 2}.get(w_dt, 4)
    w_kb_per_part = n_tiles * tile_ch * wcols_per_ch * w_elt
    # The prefetch path's 16-ch PSUM tiling has no bias-seed support; the
    # streaming fallback keeps the legacy 32-ch bias path.
    prefetch_all = w_kb_per_part <= 128 * 1024 and not has_bias
    # W tile channel sizes, alternating rings. The PE consumes channels in
    # order at ~30 ns each, so arrival order must track compute order:
    # small leading tiles let compute start early, 64-80-ch (8-10 KB-row)
    # tiles carry the bulk at good DMA efficiency, and 16-ch tail tiles
    # keep the post-stream MM+cast+out chain short. The sync ring gets
    # 272 ch vs scalar's 240 because scalar's data starts ~0.5 us later
    # (HWDGE descriptor-gen serializes the leading transfers).
    # (All-descending sizes regressed: the leading 2 MB tile delayed the
    # first MM to 26.5 us and the PE became the critical path.)
    sizes = W_SIZES
    # Output DMA coalescing (units of 16-channel PSUM tiles): big groups
    # early (fewer SWDGE issues, larger descriptors), single tiles at the
    # end (small tail granule).
    # Group ends aligned to W tile ends so each group's last cast lands
    # right after its tile arrives (unaligned ends inherit the next
    # tile's arrival as an extra wait).
    # One final issue covering the tail: serialized ~0.8 us gpsimd issue
    # costs dominate over granule size at the end.
    out_groups = [(0, 10), (10, 16), (16, 29), (29, 32)]
    with tile.TileContext(nc) as tc:
        with (
            tc.tile_pool(name="xp", bufs=1) as xp,
            tc.tile_pool(name="wpa", bufs=len(sizes)) as wpa,
            tc.tile_pool(name="op", bufs=1 if prefetch_all else N_BANKS) as op,
            tc.tile_pool(name="pp", bufs=8, space="PSUM") as pp,
        ):
            XC = xp.tile([IN_D, nparts * X_COLS], x_dt)
            if not prefetch_all:
                half = X_COLS // 2
                for ch in range(2):
                    for p in range(nparts):
                        lo = p * X_COLS + ch * half
                        xeng = nc.sync if (ch + p) % 2 == 0 else nc.scalar
                        xeng.dma_start(
                            XC[:, lo:lo + half], xc[:, lo:lo + half]
                        )
            if has_bias:
                CB = xp.tile([BANK_CH, CB_COLS], BF16)
                nc.scalar.dma_start(CB[:], cb[:])

            if prefetch_all:
                # W alternates across both HWDGE rings (one ring alone cannot
                # sustain full line rate; SDMA engines alternate descriptors
                # between rings, so both rings must carry work). Each W tile
                # is preceded ON ITS OWN RING by exactly the x slice its
                # channels need, so the arrival stream matches the PE's
                # in-order channel consumption with no cross-ring x waits.
                # Outs go to the GPSIMD SWDGE queue: in a HWDGE ring they
                # would sit in FIFO order behind the remaining W packets and
                # only drain after the whole W stream.
                # Build each ring's emission slots, then emit interleaved
                # across the two engines: the 8 global DMA sem lanes are
                # assigned in emission order, so interleaving keeps the
                # lane-reuse distance at 8 (per-engine-grouped emission
                # made sync's 5th transfer wait on its 1st and serialized
                # the issue stream ~15 us, starving the ring tail).
                tiles = {}
                slots = [[], []]  # per ring: (kind, payload)
                for r in (0, 1):
                    ring = RING_TILES[r]
                    pbase = X_POS[W_STARTS[ring[0]]]
                    csum = [0]
                    for i in ring:
                        csum.append(csum[-1] + sizes[i])
                    for ta, tb in X_PIECE_TILES:
                        tb = min(tb, len(ring))
                        lo = (pbase + csum[ta]) * B
                        hi = (pbase + csum[tb]) * B
                        slots[r].append(("x", lo, hi))
                        for i in ring[ta:tb]:
                            slots[r].append(("w", i))
                for k in range(max(len(slots[0]), len(slots[1]))):
                    for r, engine in ((0, nc.sync), (1, nc.scalar)):
                        if k >= len(slots[r]):
                            continue
                        s = slots[r][k]
                        if s[0] == "x":
                            _, lo, hi = s
                            engine.dma_start(XC[:, lo:hi], xc[:, lo:hi])
                        else:
                            i = s[1]
                            c0, tch = W_STARTS[i], sizes[i]
                            WT = wpa.tile(
                                [IN_D, tch * OUT_D], w_dt,
                                name=f"wt{i}", bufs=1,
                            )
                            engine.dma_start(
                                WT[:], wr[:, c0 * OUT_D:(c0 + tch) * OUT_D]
                            )
                            tiles[i] = (c0, c0 + tch, WT)
                tiles = [tiles[i] for i in range(len(sizes))]

                def w_slice(dl):
                    for a, b2, WT in tiles:
                        if a <= dl < b2:
                            return WT[:, (dl - a) * OUT_D:(dl - a + 1) * OUT_D]

                # One contiguous SBUF out buffer so coalesced multi-bank DMAs
                # read a single region; Tile tracks per-slice cast->dma deps.
                OBALL = op.tile([OUT_D, D_C * B], out_dt)
                # 256-col (16-channel) accumulation groups, two per physical
                # PSUM bank: 32 half-bank slots quadruple the buffer-reuse
                # distance vs 8 full banks. With only 8, the PE's
                # PSUM-recycle wait on cast g-8 formed a ~1.1 us/bank
                # lockstep convoy with the DVE that dragged compute ~9 us
                # past the end of the input DMA stream (trace-verified).
                # PSUM pool granularity is a full 2 KB bank, so allocate the
                # 8 banks once and rotate groups across half-bank slices
                # (Tile tracks deps per slice): group g -> bank g%8, half
                # (g//8)%2, so consecutive groups sit on different banks and
                # the same half recurs only at distance 16.
                ps_ch = BANK_CH // 2
                n_ps = D_C // ps_ch  # 32
                ps_tiles = [
                    pp.tile([OUT_D, BANK_CH * B], F32, name=f"ps{i}", bufs=1)
                    for i in range(8)
                ]
                for g in range(n_ps):
                    h = (g // 8) % 2
                    PS = ps_tiles[g % 8][
                        :, h * ps_ch * B:(h + 1) * ps_ch * B
                    ]
                    for j in range(ps_ch):
                        dl = g * ps_ch + j
                        xp0 = X_POS[dl]
                        nc.tensor.matmul(
                            PS[:, j * B:(j + 1) * B],
                            w_slice(dl),
                            XC[:, xp0 * B:(xp0 + 1) * B],
                            start=j == 0,
                            stop=(j == ps_ch - 1),
                        )
                    base = g * ps_ch * B
                    nc.vector.tensor_copy(
                        OBALL[:, base:base + ps_ch * B], PS[:]
                    )
                    for gi, (a, b2) in enumerate(out_groups):
                        if b2 == g + 1:
                            lo = a * ps_ch * B
                            hi = b2 * ps_ch * B
                            # Outs ride the HWDGE rings, issued strictly
                            # AFTER every W dma_start on that engine: their
                            # descriptors queue behind all W in ring FIFO,
                            # so the input stream runs with zero SWDGE
                            # contention (input crawled at <100 GB/s during
                            # SWDGE out bursts), and the cast-waits cannot
                            # delay any W descriptor generation.
                            oeng = nc.sync if gi % 2 == 0 else nc.scalar
                            oeng.dma_start(
                                outr[:, lo:hi], OBALL[:, lo:hi]
                            )
            else:
                nch = tile_ch * wcols_per_ch
                for t in range(n_tiles):
                    WT = wpa.tile([IN_D, nch], w_dt)
                    # Alternate the two HWDGE rings so W transfers overlap.
                    weng = nc.sync if t % 2 == 0 else nc.scalar
                    weng.dma_start(WT[:], wr[:, t * nch:(t + 1) * nch])
                    for h in range(banks_per_tile):
                        g = t * banks_per_tile + h
                        PS = pp.tile([OUT_D, BANK_CH * B], F32)
                        if has_bias:
                            # Seed bank: PS[o, j*16+b] = bias[g*32+j, o].
                            nc.tensor.matmul(
                                PS[:],
                                CB[:, g * OUT_D:(g + 1) * OUT_D],
                                CB[:, BN_COLS:CB_COLS],
                                start=True,
                                stop=False,
                            )
                        for j in range(BANK_CH):
                            jt = h * BANK_CH + j
                            dl = g * BANK_CH + j
                            out_sl = PS[:, j * B:(j + 1) * B]
                            whi = WT[
                                :, jt * wcols_per_ch:jt * wcols_per_ch + OUT_D
                            ]
                            xhi = XC[:, dl * B:(dl + 1) * B]
                            nc.tensor.matmul(
                                out_sl,
                                whi,
                                xhi,
                                start=(not has_bias) and j == 0,
                                stop=(mode != "mix3") and (j == BANK_CH - 1),
                            )
                            if mode == "mix3":
                                wlo = WT[
                                    :,
                                    jt * wcols_per_ch + OUT_D:
                                    (jt + 1) * wcols_per_ch,
                                ]
                                xlo = XC[
                                    :, X_COLS + dl * B:X_COLS + (dl + 1) * B
                                ]
                                nc.tensor.matmul(
                                    out_sl, whi, xlo, start=False, stop=False
                                )
                                nc.tensor.matmul(
                                    out_sl,
                                    wlo,
                                    xhi,
                                    start=False,
                                    stop=(j == BANK_CH - 1),
                                )
                        OB = op.tile([OUT_D, BANK_CH * B], out_dt)
                        nc.vector.tensor_copy(OB[:], PS[:])
                        nc.sync.dma_start(
                            outr[:, g * BANK_CH * B:(g + 1) * BANK_CH * B],
                            OB[:],
                        )

    nc.finalize()
    return nc


def _pack_x(x, sl, perm=None):
    # [b, i, dslab] -> [i, dl*16+b], channels optionally permuted into
    # the per-ring-contiguous stream order.
    xs = x[:, :, sl].transpose(1, 2, 0)  # [i, dl, b]
    if perm is not None:
        xs = xs[:, perm, :]
    return np.ascontiguousarray(xs).reshape(IN_D, X_COLS)


def _pack_bias(b, sl, eh):
    bnr = np.ascontiguousarray(
        b[sl].reshape(N_BANKS, BANK_CH, OUT_D).transpose(1, 0, 2)
    ).reshape(BANK_CH, BN_COLS)
    cbv = np.zeros((BANK_CH, CB_COLS), dtype=ml_dtypes.bfloat16)
    cbv[:, :BN_COLS] = bnr.astype(ml_dtypes.bfloat16)
    cbv[:, BN_COLS:] = eh.astype(ml_dtypes.bfloat16)
    return cbv


def _prep_core_inputs(x, W, b, mode, has_bias):
    _, w_np, _, x_np = _DT[mode]
    if mode == "f8w":
        # Power-of-2 scale pair: exact in both dtypes, cancels in the
        # product, so no on-device descale is needed.
        x = x * (1.0 / W_SCALE)
        W = W * W_SCALE
    eh = np.repeat(np.eye(BANK_CH, dtype=np.float32), B, axis=1)
    if mode == "mix3":
        bf = ml_dtypes.bfloat16
        xh = x.astype(bf)
        xl = (x - xh.astype(np.float32)).astype(bf)
        Wh = W.astype(bf)
        Wl = (W - Wh.astype(np.float32)).astype(bf)
    in_maps = []
    for c in range(NCORES):
        sl = slice(c * D_C, (c + 1) * D_C)
        if mode == "mix3":
            xcv = np.concatenate(
                [_pack_x(xh.astype(np.float32), sl), _pack_x(xl.astype(np.float32), sl)],
                axis=1,
            ).astype(bf)
            wrv = np.ascontiguousarray(
                np.stack(
                    [Wh[sl].transpose(1, 0, 2), Wl[sl].transpose(1, 0, 2)],
                    axis=2,
                )
            ).reshape(IN_D, D_C * 2 * OUT_D)
            m = {"xc": xcv, "wr": wrv}
            if has_bias:
                m["cb"] = _pack_bias(b, sl, eh)
            in_maps.append(m)
            continue
        # The prefetch path (bf16/f8w, no bias) consumes per-ring-permuted
        # x; the streaming fallback uses plain channel order.
        use_perm = mode in ("bf16", "f8w") and not has_bias
        xr = _pack_x(x, sl, X_PERM if use_perm else None).astype(
            x_np, copy=False
        )
        wrv = (
            np.ascontiguousarray(W[sl].transpose(1, 0, 2))
            .reshape(IN_D, D_C * OUT_D)
            .astype(w_np, copy=False)
        )
        m = {"xc": xr, "wr": wrv}
        if has_bias:
            m["cb"] = _pack_bias(b, sl, eh)
        in_maps.append(m)
    return in_maps


def run(inputs, trace=False, mode=None):
    mode = mode or MM_DTYPE
    x = np.asarray(inputs["x"], dtype=np.float32)
    W = np.asarray(inputs["W"], dtype=np.float32)
    b = np.asarray(inputs["b"], dtype=np.float32)
    has_bias = bool(np.any(b))
    key = (mode, has_bias)
    if key not in _cached:
        _cached[key] = _build(mode, has_bias)
    in_maps = _prep_core_inputs(x, W, b, mode, has_bias)
    res = run_bass_kernel_spmd(
        _cached[key], in_maps, core_ids=list(range(NCORES)), trace=trace
    )
    out = np.empty((B, OUT_D, D_TOTAL), dtype=np.float32)
    for c in range(NCORES):
        sl = slice(c * D_C, (c + 1) * D_C)
        out[:, :, sl] = (
            np.asarray(res.results[c]["outr"])
            .astype(np.float32)
            .reshape(OUT_D, D_C, B)
            .transpose(2, 0, 1)
        )
    return out, res


def kernel(**inputs):
    out, _ = run(inputs)
    return out



# revision 48
# speedup vs baseline: 1.0050x; 1.0050x over previous
"""Trainium2 Bass kernel for grouped per-channel linears (nn_GroupedLinearsAdvanced).

Math: out[b, o, d] = sum_i x[b, i, d] * W[d, i, o] + bias[d, o]
with x: [16, 128, 4096] f32, W: [4096, 128, 128] f32, bias: [4096, 128] f32,
out: [16, 128, 4096] f32.

Sharding: channel dim D=4096 split into 8 contiguous slabs of 512 channels,
one per NeuronCore; x slices replicated per-slab, no cross-device reduction.

Per-core dataflow (DMA-bound; ~20 MB of HBM traffic per core at ~26 GB/s
per SDMA engine x 16 engines):
  - host pre-permutes inputs so every DMA moves long contiguous
    per-partition runs; x and W are cast to bf16 on host (halves W traffic
    vs fp32; rel-err ~3e-3 against the fp32 reference),
  - x slab resident in SBUF: layout [i, dl*16+b]; its two 1 MB chunks go
    one per HWDGE ring so both rings carry equal bytes (9 MB each),
  - the whole 16 MB W slab is prefetched through SBUF in 8 x 2 MB tiles,
    all dma_starts issued up front, tiles alternating between the sync and
    scalar HWDGE rings (SDMA engines alternate descriptors between rings;
    a single ring cannot sustain line rate),
  - per channel: one matmul  PS[o, b] = W_d.T @ x_d.T  (lhsT = W_d),
    32 channels accumulate side-by-side into one 512-f32 PSUM bank,
  - bias (when nonzero) seeds each PSUM bank via a bf16 one-hot expansion
    matmul: PS[o, j*16+b] = bias[g*32+j, o] = (BN_g).T @ E; skipped when
    the supplied bias is identically zero,
  - DVE evacuates each bank to SBUF casting f32 -> bf16 (halves out DMA),
  - outs stream to HBM via the GPSIMD SWDGE queue so they never queue in
    a HWDGE ring FIFO behind undrained W packets; host casts back to f32.

MM_DTYPE picks the tensor-engine path for x/W:
  "f32"  — exact fp32 (hardware runs 2 half-speed passes per matmul),
  "f32r" — same fp32 bytes, single-pass reduced-precision PE mode,
  "bf16" — host-side cast, halves DMA traffic, single-pass matmuls + FWL,
  "mix3" — bf16 hi+lo split, fp32-class accuracy at fp32 bytes (slow path).
"""

import ml_dtypes
import numpy as np

from concourse import bacc, mybir, tile
from concourse.bass_utils import run_bass_kernel_spmd

B = 16           # batch
IN_D = 128       # contraction dim (SBUF partitions)
OUT_D = 128      # per-channel output dim
D_TOTAL = 4096   # channels
NCORES = 8
D_C = D_TOTAL // NCORES      # 512 channels per core
BANK_CH = 32                 # channels per PSUM bank (32*16 = 512 fp32 = 1 bank)
N_BANKS = D_C // BANK_CH     # 16

X_COLS = D_C * B                 # 8192
BN_COLS = N_BANKS * OUT_D        # 2048
EH_COLS = BANK_CH * B            # 512
CB_COLS = BN_COLS + EH_COLS      # bias + one-hot constant tensor

F32 = mybir.dt.float32
BF16 = mybir.dt.bfloat16

MM_DTYPE = "f8w"

F8E3 = mybir.dt.float8e3

# (w_dtype_mybir, w_dtype_np, x_dtype_mybir, x_dtype_np)
# f8w: W quantized to fp8-e3m4 (TRN FP8_EXP3, 4 mantissa bits) with a
# global power-of-2 scale (W*128, absmax 6.9 << 15.5 so no clipping);
# x carries the inverse scale (x/128, exact in bf16). Halves W HBM
# traffic vs bf16; numpy-simulated rel-err 1.22e-2 vs the fp32
# reference (threshold 2e-2). The PE allows mixed fp8-weight x
# bf16-moving matmuls (only fp32 requires matching operand dtypes).
W_SCALE = 128.0
_DT = {
    "f32": (F32, np.float32, F32, np.float32),
    "f32r": (mybir.dt.float32r, np.float32, mybir.dt.float32r, np.float32),
    "bf16": (BF16, ml_dtypes.bfloat16, BF16, ml_dtypes.bfloat16),
    "f8w": (F8E3, ml_dtypes.float8_e3m4, BF16, ml_dtypes.bfloat16),
    # mix3: W and x split into bf16 hi+lo parts; 3 single-pass matmuls
    # per channel (hi*hi + lo*hi + hi*lo) recover ~1e-5 accuracy while
    # keeping bf16 tensor-engine throughput. Same HBM bytes as fp32.
    "mix3": (BF16, ml_dtypes.bfloat16, BF16, ml_dtypes.bfloat16),
}

# --- stream layout shared by host packing and kernel build ---
# W tiles alternate rings (even index -> sync). x is packed per-ring
# contiguous (all sync tiles' channels first, then scalar's) so each
# ring's x moves as a few large-row transfers interleaved with its W
# tiles instead of one 2 KB-row sliver per tile.
W_SIZES = [32, 32, 64, 64, 64, 64, 80, 64, 16, 16, 16]
assert sum(W_SIZES) == D_C
# Per-ring x piece boundaries, counted in that ring's tile list: piece 0
# covers tile 0, piece 1 tiles 1-2, piece 2 the rest. Emitted before the
# first W tile each covers.
X_PIECE_TILES = [(0, 1), (1, 3), (3, 99)]


def _stream_layout():
    ring_tiles = [[], []]  # tile indices per ring
    starts = []
    c0 = 0
    for i, tch in enumerate(W_SIZES):
        ring_tiles[i % 2].append(i)
        starts.append(c0)
        c0 += tch
    perm = []  # permuted channel order: sync rings' channels, then scalar
    for r in (0, 1):
        for i in ring_tiles[r]:
            perm.extend(range(starts[i], starts[i] + W_SIZES[i]))
    pos = [0] * D_C  # channel -> position in permuted x
    for p, ch in enumerate(perm):
        pos[ch] = p
    return ring_tiles, starts, perm, pos


RING_TILES, W_STARTS, X_PERM, X_POS = _stream_layout()

_cached = {}


def _build(mode, has_bias):
    w_dt, _, x_dt, _ = _DT[mode]
    nparts = 2 if mode == "mix3" else 1  # hi/lo operand copies
    out_dt = BF16 if mode in ("bf16", "f8w") else F32
    nc = bacc.Bacc()
    xc = nc.dram_tensor("xc", [IN_D, nparts * X_COLS], x_dt, kind="ExternalInput")
    wr = nc.dram_tensor(
        "wr", [IN_D, nparts * D_C * OUT_D], w_dt, kind="ExternalInput"
    )
    if has_bias:
        cb = nc.dram_tensor("cb", [BANK_CH, CB_COLS], BF16, kind="ExternalInput")
    outr = nc.dram_tensor("outr", [OUT_D, D_C * B], out_dt, kind="ExternalOutput")

    # Streaming-path (non-prefetch) tile size; the prefetch path below uses
    # `sizes` instead.
    tile_ch = 32 // nparts
    n_tiles = D_C // tile_ch
    banks_per_tile = max(1, tile_ch // BANK_CH)
    wcols_per_ch = nparts * OUT_D
    # Prefetch the whole W slab when it fits in SBUF (bf16: 8 tiles x
    # 16 KB/partition = 128 KB). All W dma_starts issue up front, so a
    # later out-DMA's semaphore wait on the sync ring can never delay a
    # W transfer (HWDGE rings are FIFO per issuing engine).
    w_elt = {F8E3: 1, BF16: 2}.get(w_dt, 4)
    w_kb_per_part = n_tiles * tile_ch * wcols_per_ch * w_elt
    # The prefetch path's 16-ch PSUM tiling has no bias-seed support; the
    # streaming fallback keeps the legacy 32-ch bias path.
    prefetch_all = w_kb_per_part <= 128 * 1024 and not has_bias
    # W tile channel sizes, alternating rings. The PE consumes channels in
    # order at ~30 ns each, so arrival order must track compute order:
    # small leading tiles let compute start early, 64-80-ch (8-10 KB-row)
    # tiles carry the bulk at good DMA efficiency, and 16-ch tail tiles
    # keep the post-stream MM+cast+out chain short. The sync ring gets
    # 272 ch vs scalar's 240 because scalar's data starts ~0.5 us later
    # (HWDGE descriptor-gen serializes the leading transfers).
    # (All-descending sizes regressed: the leading 2 MB tile delayed the
    # first MM to 26.5 us and the PE became the critical path.)
    sizes = W_SIZES
    # Output DMA coalescing (units of 16-channel PSUM tiles): big groups
    # early (fewer SWDGE issues, larger descriptors), single tiles at the
    # end (small tail granule).
    # Group ends aligned to W tile ends so each group's last cast lands
    # right after its tile arrives (unaligned ends inherit the next
    # tile's arrival as an extra wait).
    # One final issue covering the tail: serialized ~0.8 us gpsimd issue
    # costs dominate over granule size at the end.
    out_groups = [(0, 8), (8, 16), (16, 25), (25, 29), (29, 31), (31, 32)]
    with tile.TileContext(nc) as tc:
        with (
            tc.tile_pool(name="xp", bufs=1) as xp,
            tc.tile_pool(name="wpa", bufs=len(sizes)) as wpa,
            tc.tile_pool(name="op", bufs=1 if prefetch_all else N_BANKS) as op,
            tc.tile_pool(name="pp", bufs=8, space="PSUM") as pp,
        ):
            XC = xp.tile([IN_D, nparts * X_COLS], x_dt)
            if not prefetch_all:
                half = X_COLS // 2
                for ch in range(2):
                    for p in range(nparts):
                        lo = p * X_COLS + ch * half
                        xeng = nc.sync if (ch + p) % 2 == 0 else nc.scalar
                        xeng.dma_start(
                            XC[:, lo:lo + half], xc[:, lo:lo + half]
                        )
            if has_bias:
                CB = xp.tile([BANK_CH, CB_COLS], BF16)
                nc.scalar.dma_start(CB[:], cb[:])

            if prefetch_all:
                # W alternates across both HWDGE rings (one ring alone cannot
                # sustain full line rate; SDMA engines alternate descriptors
                # between rings, so both rings must carry work). Each W tile
                # is preceded ON ITS OWN RING by exactly the x slice its
                # channels need, so the arrival stream matches the PE's
                # in-order channel consumption with no cross-ring x waits.
                # Outs go to the GPSIMD SWDGE queue: in a HWDGE ring they
                # would sit in FIFO order behind the remaining W packets and
                # only drain after the whole W stream.
                # Build each ring's emission slots, then emit interleaved
                # across the two engines: the 8 global DMA sem lanes are
                # assigned in emission order, so interleaving keeps the
                # lane-reuse distance at 8 (per-engine-grouped emission
                # made sync's 5th transfer wait on its 1st and serialized
                # the issue stream ~15 us, starving the ring tail).
                tiles = {}
                slots = [[], []]  # per ring: (kind, payload)
                for r in (0, 1):
                    ring = RING_TILES[r]
                    pbase = X_POS[W_STARTS[ring[0]]]
                    csum = [0]
                    for i in ring:
                        csum.append(csum[-1] + sizes[i])
                    for ta, tb in X_PIECE_TILES:
                        tb = min(tb, len(ring))
                        lo = (pbase + csum[ta]) * B
                        hi = (pbase + csum[tb]) * B
                        slots[r].append(("x", lo, hi))
                        for i in ring[ta:tb]:
                            slots[r].append(("w", i))
                for k in range(max(len(slots[0]), len(slots[1]))):
                    for r, engine in ((0, nc.sync), (1, nc.scalar)):
                        if k >= len(slots[r]):
                            continue
                        s = slots[r][k]
                        if s[0] == "x":
                            _, lo, hi = s
                            engine.dma_start(XC[:, lo:hi], xc[:, lo:hi])
                        else:
                            i = s[1]
                            c0, tch = W_STARTS[i], sizes[i]
                            WT = wpa.tile(
                                [IN_D, tch * OUT_D], w_dt,
                                name=f"wt{i}", bufs=1,
                            )
                            engine.dma_start(
                                WT[:], wr[:, c0 * OUT_D:(c0 + tch) * OUT_D]
                            )
                            tiles[i] = (c0, c0 + tch, WT)
                tiles = [tiles[i] for i in range(len(sizes))]

                def w_slice(dl):
                    for a, b2, WT in tiles:
                        if a <= dl < b2:
                            return WT[:, (dl - a) * OUT_D:(dl - a + 1) * OUT_D]

                # One contiguous SBUF out buffer so coalesced multi-bank DMAs
                # read a single region; Tile tracks per-slice cast->dma deps.
                OBALL = op.tile([OUT_D, D_C * B], out_dt)
                # 256-col (16-channel) accumulation groups, two per physical
                # PSUM bank: 32 half-bank slots quadruple the buffer-reuse
                # distance vs 8 full banks. With only 8, the PE's
                # PSUM-recycle wait on cast g-8 formed a ~1.1 us/bank
                # lockstep convoy with the DVE that dragged compute ~9 us
                # past the end of the input DMA stream (trace-verified).
                # PSUM pool granularity is a full 2 KB bank, so allocate the
                # 8 banks once and rotate groups across half-bank slices
                # (Tile tracks deps per slice): group g -> bank g%8, half
                # (g//8)%2, so consecutive groups sit on different banks and
                # the same half recurs only at distance 16.
                ps_ch = BANK_CH // 2
                n_ps = D_C // ps_ch  # 32
                ps_tiles = [
                    pp.tile([OUT_D, BANK_CH * B], F32, name=f"ps{i}", bufs=1)
                    for i in range(8)
                ]
                for g in range(n_ps):
                    h = (g // 8) % 2
                    PS = ps_tiles[g % 8][
                        :, h * ps_ch * B:(h + 1) * ps_ch * B
                    ]
                    for j in range(ps_ch):
                        dl = g * ps_ch + j
                        xp0 = X_POS[dl]
                        nc.tensor.matmul(
                            PS[:, j * B:(j + 1) * B],
                            w_slice(dl),
                            XC[:, xp0 * B:(xp0 + 1) * B],
                            start=j == 0,
                            stop=(j == ps_ch - 1),
                        )
                    base = g * ps_ch * B
                    nc.vector.tensor_copy(
                        OBALL[:, base:base + ps_ch * B], PS[:]
                    )
                    for gi, (a, b2) in enumerate(out_groups):
                        if b2 == g + 1:
                            lo = a * ps_ch * B
                            hi = b2 * ps_ch * B
                            # Outs ride the HWDGE rings, issued strictly
                            # AFTER every W dma_start on that engine: their
                            # descriptors queue behind all W in ring FIFO,
                            # so the input stream runs with zero SWDGE
                            # contention (input crawled at <100 GB/s during
                            # SWDGE out bursts), and the cast-waits cannot
                            # delay any W descriptor generation.
                            if gi < 2:
                                oeng = nc.gpsimd
                            else:
                                oeng = nc.sync if gi % 2 == 0 else nc.scalar
                            oeng.dma_start(
                                outr[:, lo:hi], OBALL[:, lo:hi]
                            )
            else:
                nch = tile_ch * wcols_per_ch
                for t in range(n_tiles):
                    WT = wpa.tile([IN_D, nch], w_dt)
                    # Alternate the two HWDGE rings so W transfers overlap.
                    weng = nc.sync if t % 2 == 0 else nc.scalar
                    weng.dma_start(WT[:], wr[:, t * nch:(t + 1) * nch])
                    for h in range(banks_per_tile):
                        g = t * banks_per_tile + h
                        PS = pp.tile([OUT_D, BANK_CH * B], F32)
                        if has_bias:
                            # Seed bank: PS[o, j*16+b] = bias[g*32+j, o].
                            nc.tensor.matmul(
                                PS[:],
                                CB[:, g * OUT_D:(g + 1) * OUT_D],
                                CB[:, BN_COLS:CB_COLS],
                                start=True,
                                stop=False,
                            )
                        for j in range(BANK_CH):
                            jt = h * BANK_CH + j
                            dl = g * BANK_CH + j
                            out_sl = PS[:, j * B:(j + 1) * B]
                            whi = WT[
                                :, jt * wcols_per_ch:jt * wcols_per_ch + OUT_D
                            ]
                            xhi = XC[:, dl * B:(dl + 1) * B]
                            nc.tensor.matmul(
                                out_sl,
                                whi,
                                xhi,
                                start=(not has_bias) and j == 0,
                                stop=(mode != "mix3") and (j == BANK_CH - 1),
                            )
                            if mode == "mix3":
                                wlo = WT[
                                    :,
                                    jt * wcols_per_ch + OUT_D:
                                    (jt + 1) * wcols_per_ch,
                                ]
                                xlo = XC[
                                    :, X_COLS + dl * B:X_COLS + (dl + 1) * B
                                ]
                                nc.tensor.matmul(
                                    out_sl, whi, xlo, start=False, stop=False
                                )
                                nc.tensor.matmul(
                                    out_sl,
                                    wlo,
                                    xhi,
                                    start=False,
                                    stop=(j == BANK_CH - 1),
                                )
                        OB = op.tile([OUT_D, BANK_CH * B], out_dt)
                        nc.vector.tensor_copy(OB[:], PS[:])
                        nc.sync.dma_start(
                            outr[:, g * BANK_CH * B:(g + 1) * BANK_CH * B],
                            OB[:],
                        )

    nc.finalize()
    return nc


def _pack_x(x, sl, perm=None):
    # [b, i, dslab] -> [i, dl*16+b], channels optionally permuted into
    # the per-ring-contiguous stream order.
    xs = x[:, :, sl].transpose(1, 2, 0)  # [i, dl, b]
    if perm is not None:
        xs = xs[:, perm, :]
    return np.ascontiguousarray(xs).reshape(IN_D, X_COLS)


def _pack_bias(b, sl, eh):
    bnr = np.ascontiguousarray(
        b[sl].reshape(N_BANKS, BANK_CH, OUT_D).transpose(1, 0, 2)
    ).reshape(BANK_CH, BN_COLS)
    cbv = np.zeros((BANK_CH, CB_COLS), dtype=ml_dtypes.bfloat16)
    cbv[:, :BN_COLS] = bnr.astype(ml_dtypes.bfloat16)
    cbv[:, BN_COLS:] = eh.astype(ml_dtypes.bfloat16)
    return cbv


def _prep_core_inputs(x, W, b, mode, has_bias):
    _, w_np, _, x_np = _DT[mode]
    if mode == "f8w":
        # Power-of-2 scale pair: exact in both dtypes, cancels in the
        # product, so no on-device descale is needed.
        x = x * (1.0 / W_SCALE)
        W = W * W_SCALE
    eh = np.repeat(np.eye(BANK_CH, dtype=np.float32), B, axis=1)
    if mode == "mix3":
        bf = ml_dtypes.bfloat16
        xh = x.astype(bf)
        xl = (x - xh.astype(np.float32)).astype(bf)
        Wh = W.astype(bf)
        Wl = (W - Wh.astype(np.float32)).astype(bf)
    in_maps = []
    for c in range(NCORES):
        sl = slice(c * D_C, (c + 1) * D_C)
        if mode == "mix3":
            xcv = np.concatenate(
                [_pack_x(xh.astype(np.float32), sl), _pack_x(xl.astype(np.float32), sl)],
                axis=1,
            ).astype(bf)
            wrv = np.ascontiguousarray(
                np.stack(
                    [Wh[sl].transpose(1, 0, 2), Wl[sl].transpose(1, 0, 2)],
                    axis=2,
                )
            ).reshape(IN_D, D_C * 2 * OUT_D)
            m = {"xc": xcv, "wr": wrv}
            if has_bias:
                m["cb"] = _pack_bias(b, sl, eh)
            in_maps.append(m)
            continue
        # The prefetch path (bf16/f8w, no bias) consumes per-ring-permuted
        # x; the streaming fallback uses plain channel order.
        use_perm = mode in ("bf16", "f8w") and not has_bias
        xr = _pack_x(x, sl, X_PERM if use_perm else None).astype(
            x_np, copy=False
        )
        wrv = (
            np.ascontiguousarray(W[sl].transpose(1, 0, 2))
            .reshape(IN_D, D_C * OUT_D)
            .astype(w_np, copy=False)
        )
        m = {"xc": xr, "wr": wrv}
        if has_bias:
            m["cb"] = _pack_bias(b, sl, eh)
        in_maps.append(m)
    return in_maps


def run(inputs, trace=False, mode=None):
    mode = mode or MM_DTYPE
    x = np.asarray(inputs["x"], dtype=np.float32)
    W = np.asarray(inputs["W"], dtype=np.float32)
    b = np.asarray(inputs["b"], dtype=np.float32)
    has_bias = bool(np.any(b))
    key = (mode, has_bias)
    if key not in _cached:
        _cached[key] = _build(mode, has_bias)
    in_maps = _prep_core_inputs(x, W, b, mode, has_bias)
    res = run_bass_kernel_spmd(
        _cached[key], in_maps, core_ids=list(range(NCORES)), trace=trace
    )
    out = np.empty((B, OUT_D, D_TOTAL), dtype=np.float32)
    for c in range(NCORES):
        sl = slice(c * D_C, (c + 1) * D_C)
        out[:, :, sl] = (
            np.asarray(res.results[c]["outr"])
            .astype(np.float32)
            .reshape(OUT_D, D_C, B)
            .transpose(2, 0, 1)
        )
    return out, res


def kernel(**inputs):
    out, _ = run(inputs)
    return out



# revision 49
# speedup vs baseline: 1.0508x; 1.0456x over previous
"""Trainium2 Bass kernel for grouped per-channel linears (nn_GroupedLinearsAdvanced).

Math: out[b, o, d] = sum_i x[b, i, d] * W[d, i, o] + bias[d, o]
with x: [16, 128, 4096] f32, W: [4096, 128, 128] f32, bias: [4096, 128] f32,
out: [16, 128, 4096] f32.

Sharding: channel dim D=4096 split into 8 contiguous slabs of 512 channels,
one per NeuronCore; x slices replicated per-slab, no cross-device reduction.

Per-core dataflow (DMA-bound; ~20 MB of HBM traffic per core at ~26 GB/s
per SDMA engine x 16 engines):
  - host pre-permutes inputs so every DMA moves long contiguous
    per-partition runs; x and W are cast to bf16 on host (halves W traffic
    vs fp32; rel-err ~3e-3 against the fp32 reference),
  - x slab resident in SBUF: layout [i, dl*16+b]; its two 1 MB chunks go
    one per HWDGE ring so both rings carry equal bytes (9 MB each),
  - the whole 16 MB W slab is prefetched through SBUF in 8 x 2 MB tiles,
    all dma_starts issued up front, tiles alternating between the sync and
    scalar HWDGE rings (SDMA engines alternate descriptors between rings;
    a single ring cannot sustain line rate),
  - per channel: one matmul  PS[o, b] = W_d.T @ x_d.T  (lhsT = W_d),
    32 channels accumulate side-by-side into one 512-f32 PSUM bank,
  - bias (when nonzero) seeds each PSUM bank via a bf16 one-hot expansion
    matmul: PS[o, j*16+b] = bias[g*32+j, o] = (BN_g).T @ E; skipped when
    the supplied bias is identically zero,
  - DVE evacuates each bank to SBUF casting f32 -> bf16 (halves out DMA),
  - outs stream to HBM via the GPSIMD SWDGE queue so they never queue in
    a HWDGE ring FIFO behind undrained W packets; host casts back to f32.

MM_DTYPE picks the tensor-engine path for x/W:
  "f32"  — exact fp32 (hardware runs 2 half-speed passes per matmul),
  "f32r" — same fp32 bytes, single-pass reduced-precision PE mode,
  "bf16" — host-side cast, halves DMA traffic, single-pass matmuls + FWL,
  "mix3" — bf16 hi+lo split, fp32-class accuracy at fp32 bytes (slow path).
"""

import ml_dtypes
import numpy as np

from concourse import bacc, mybir, tile
from concourse.bass_utils import run_bass_kernel_spmd

B = 16           # batch
IN_D = 128       # contraction dim (SBUF partitions)
OUT_D = 128      # per-channel output dim
D_TOTAL = 4096   # channels
NCORES = 8
D_C = D_TOTAL // NCORES      # 512 channels per core
BANK_CH = 32                 # channels per PSUM bank (32*16 = 512 fp32 = 1 bank)
N_BANKS = D_C // BANK_CH     # 16

X_COLS = D_C * B                 # 8192
BN_COLS = N_BANKS * OUT_D        # 2048
EH_COLS = BANK_CH * B            # 512
CB_COLS = BN_COLS + EH_COLS      # bias + one-hot constant tensor

F32 = mybir.dt.float32
BF16 = mybir.dt.bfloat16

MM_DTYPE = "f8w"

F8E3 = mybir.dt.float8e3

# (w_dtype_mybir, w_dtype_np, x_dtype_mybir, x_dtype_np)
# f8w: W quantized to fp8-e3m4 (TRN FP8_EXP3, 4 mantissa bits) with a
# global power-of-2 scale (W*128, absmax 6.9 << 15.5 so no clipping);
# x carries the inverse scale (x/128, exact in bf16). Halves W HBM
# traffic vs bf16; numpy-simulated rel-err 1.22e-2 vs the fp32
# reference (threshold 2e-2). The PE allows mixed fp8-weight x
# bf16-moving matmuls (only fp32 requires matching operand dtypes).
W_SCALE = 128.0
_DT = {
    "f32": (F32, np.float32, F32, np.float32),
    "f32r": (mybir.dt.float32r, np.float32, mybir.dt.float32r, np.float32),
    "bf16": (BF16, ml_dtypes.bfloat16, BF16, ml_dtypes.bfloat16),
    "f8w": (F8E3, ml_dtypes.float8_e3m4, BF16, ml_dtypes.bfloat16),
    # mix3: W and x split into bf16 hi+lo parts; 3 single-pass matmuls
    # per channel (hi*hi + lo*hi + hi*lo) recover ~1e-5 accuracy while
    # keeping bf16 tensor-engine throughput. Same HBM bytes as fp32.
    "mix3": (BF16, ml_dtypes.bfloat16, BF16, ml_dtypes.bfloat16),
}

# --- stream layout shared by host packing and kernel build ---
# W tiles alternate rings (even index -> sync). x is packed per-ring
# contiguous (all sync tiles' channels first, then scalar's) so each
# ring's x moves as a few large-row transfers interleaved with its W
# tiles instead of one 2 KB-row sliver per tile.
W_SIZES = [32, 32, 64, 64, 64, 64, 48, 48, 32, 32, 16, 16]
assert sum(W_SIZES) == D_C
# Per-ring x piece boundaries, counted in that ring's tile list: piece 0
# covers tile 0, piece 1 tiles 1-2, piece 2 the rest. Emitted before the
# first W tile each covers.
X_PIECE_TILES = [(0, 1), (1, 3), (3, 99)]


def _stream_layout():
    ring_tiles = [[], []]  # tile indices per ring
    starts = []
    c0 = 0
    for i, tch in enumerate(W_SIZES):
        ring_tiles[i % 2].append(i)
        starts.append(c0)
        c0 += tch
    perm = []  # permuted channel order: sync rings' channels, then scalar
    for r in (0, 1):
        for i in ring_tiles[r]:
            perm.extend(range(starts[i], starts[i] + W_SIZES[i]))
    pos = [0] * D_C  # channel -> position in permuted x
    for p, ch in enumerate(perm):
        pos[ch] = p
    return ring_tiles, starts, perm, pos


RING_TILES, W_STARTS, X_PERM, X_POS = _stream_layout()

_cached = {}


def _build(mode, has_bias):
    w_dt, _, x_dt, _ = _DT[mode]
    nparts = 2 if mode == "mix3" else 1  # hi/lo operand copies
    out_dt = BF16 if mode in ("bf16", "f8w") else F32
    nc = bacc.Bacc()
    xc = nc.dram_tensor("xc", [IN_D, nparts * X_COLS], x_dt, kind="ExternalInput")
    wr = nc.dram_tensor(
        "wr", [IN_D, nparts * D_C * OUT_D], w_dt, kind="ExternalInput"
    )
    if has_bias:
        cb = nc.dram_tensor("cb", [BANK_CH, CB_COLS], BF16, kind="ExternalInput")
    outr = nc.dram_tensor("outr", [OUT_D, D_C * B], out_dt, kind="ExternalOutput")

    # Streaming-path (non-prefetch) tile size; the prefetch path below uses
    # `sizes` instead.
    tile_ch = 32 // nparts
    n_tiles = D_C // tile_ch
    banks_per_tile = max(1, tile_ch // BANK_CH)
    wcols_per_ch = nparts * OUT_D
    # Prefetch the whole W slab when it fits in SBUF (bf16: 8 tiles x
    # 16 KB/partition = 128 KB). All W dma_starts issue up front, so a
    # later out-DMA's semaphore wait on the sync ring can never delay a
    # W transfer (HWDGE rings are FIFO per issuing engine).
    w_elt = {F8E3: 1, BF16: 2}.get(w_dt, 4)
    w_kb_per_part = n_tiles * tile_ch * wcols_per_ch * w_elt
    # The prefetch path's 16-ch PSUM tiling has no bias-seed support; the
    # streaming fallback keeps the legacy 32-ch bias path.
    prefetch_all = w_kb_per_part <= 128 * 1024 and not has_bias
    # W tile channel sizes, alternating rings. The PE consumes channels in
    # order at ~30 ns each, so arrival order must track compute order:
    # small leading tiles let compute start early, 64-80-ch (8-10 KB-row)
    # tiles carry the bulk at good DMA efficiency, and 16-ch tail tiles
    # keep the post-stream MM+cast+out chain short. The sync ring gets
    # 272 ch vs scalar's 240 because scalar's data starts ~0.5 us later
    # (HWDGE descriptor-gen serializes the leading transfers).
    # (All-descending sizes regressed: the leading 2 MB tile delayed the
    # first MM to 26.5 us and the PE became the critical path.)
    sizes = W_SIZES
    # Output DMA coalescing (units of 16-channel PSUM tiles): big groups
    # early (fewer SWDGE issues, larger descriptors), single tiles at the
    # end (small tail granule).
    # Group ends aligned to W tile ends so each group's last cast lands
    # right after its tile arrives (unaligned ends inherit the next
    # tile's arrival as an extra wait).
    # One final issue covering the tail: serialized ~0.8 us gpsimd issue
    # costs dominate over granule size at the end.
    out_groups = [(0, 8), (8, 16), (16, 23), (23, 28), (28, 31), (31, 32)]
    with tile.TileContext(nc) as tc:
        with (
            tc.tile_pool(name="xp", bufs=1) as xp,
            tc.tile_pool(name="wpa", bufs=len(sizes)) as wpa,
            tc.tile_pool(name="op", bufs=1 if prefetch_all else N_BANKS) as op,
            tc.tile_pool(name="pp", bufs=8, space="PSUM") as pp,
        ):
            XC = xp.tile([IN_D, nparts * X_COLS], x_dt)
            if not prefetch_all:
                half = X_COLS // 2
                for ch in range(2):
                    for p in range(nparts):
                        lo = p * X_COLS + ch * half
                        xeng = nc.sync if (ch + p) % 2 == 0 else nc.scalar
                        xeng.dma_start(
                            XC[:, lo:lo + half], xc[:, lo:lo + half]
                        )
            if has_bias:
                CB = xp.tile([BANK_CH, CB_COLS], BF16)
                nc.scalar.dma_start(CB[:], cb[:])

            if prefetch_all:
                # W alternates across both HWDGE rings (one ring alone cannot
                # sustain full line rate; SDMA engines alternate descriptors
                # between rings, so both rings must carry work). Each W tile
                # is preceded ON ITS OWN RING by exactly the x slice its
                # channels need, so the arrival stream matches the PE's
                # in-order channel consumption with no cross-ring x waits.
                # Outs go to the GPSIMD SWDGE queue: in a HWDGE ring they
                # would sit in FIFO order behind the remaining W packets and
                # only drain after the whole W stream.
                # Build each ring's emission slots, then emit interleaved
                # across the two engines: the 8 global DMA sem lanes are
                # assigned in emission order, so interleaving keeps the
                # lane-reuse distance at 8 (per-engine-grouped emission
                # made sync's 5th transfer wait on its 1st and serialized
                # the issue stream ~15 us, starving the ring tail).
                tiles = {}
                slots = [[], []]  # per ring: (kind, payload)
                for r in (0, 1):
                    ring = RING_TILES[r]
                    pbase = X_POS[W_STARTS[ring[0]]]
                    csum = [0]
                    for i in ring:
                        csum.append(csum[-1] + sizes[i])
                    for ta, tb in X_PIECE_TILES:
                        tb = min(tb, len(ring))
                        lo = (pbase + csum[ta]) * B
                        hi = (pbase + csum[tb]) * B
                        slots[r].append(("x", lo, hi))
                        for i in ring[ta:tb]:
                            slots[r].append(("w", i))
                for k in range(max(len(slots[0]), len(slots[1]))):
                    for r, engine in ((0, nc.sync), (1, nc.scalar)):
                        if k >= len(slots[r]):
                            continue
                        s = slots[r][k]
                        if s[0] == "x":
                            _, lo, hi = s
                            engine.dma_start(XC[:, lo:hi], xc[:, lo:hi])
                        else:
                            i = s[1]
                            c0, tch = W_STARTS[i], sizes[i]
                            WT = wpa.tile(
                                [IN_D, tch * OUT_D], w_dt,
                                name=f"wt{i}", bufs=1,
                            )
                            engine.dma_start(
                                WT[:], wr[:, c0 * OUT_D:(c0 + tch) * OUT_D]
                            )
                            tiles[i] = (c0, c0 + tch, WT)
                tiles = [tiles[i] for i in range(len(sizes))]

                def w_slice(dl):
                    for a, b2, WT in tiles:
                        if a <= dl < b2:
                            return WT[:, (dl - a) * OUT_D:(dl - a + 1) * OUT_D]

                # One contiguous SBUF out buffer so coalesced multi-bank DMAs
                # read a single region; Tile tracks per-slice cast->dma deps.
                OBALL = op.tile([OUT_D, D_C * B], out_dt)
                # 256-col (16-channel) accumulation groups, two per physical
                # PSUM bank: 32 half-bank slots quadruple the buffer-reuse
                # distance vs 8 full banks. With only 8, the PE's
                # PSUM-recycle wait on cast g-8 formed a ~1.1 us/bank
                # lockstep convoy with the DVE that dragged compute ~9 us
                # past the end of the input DMA stream (trace-verified).
                # PSUM pool granularity is a full 2 KB bank, so allocate the
                # 8 banks once and rotate groups across half-bank slices
                # (Tile tracks deps per slice): group g -> bank g%8, half
                # (g//8)%2, so consecutive groups sit on different banks and
                # the same half recurs only at distance 16.
                ps_ch = BANK_CH // 2
                n_ps = D_C // ps_ch  # 32
                ps_tiles = [
                    pp.tile([OUT_D, BANK_CH * B], F32, name=f"ps{i}", bufs=1)
                    for i in range(8)
                ]
                for g in range(n_ps):
                    h = (g // 8) % 2
                    PS = ps_tiles[g % 8][
                        :, h * ps_ch * B:(h + 1) * ps_ch * B
                    ]
                    for j in range(ps_ch):
                        dl = g * ps_ch + j
                        xp0 = X_POS[dl]
                        nc.tensor.matmul(
                            PS[:, j * B:(j + 1) * B],
                            w_slice(dl),
                            XC[:, xp0 * B:(xp0 + 1) * B],
                            start=j == 0,
                            stop=(j == ps_ch - 1),
                        )
                    base = g * ps_ch * B
                    nc.vector.tensor_copy(
                        OBALL[:, base:base + ps_ch * B], PS[:]
                    )
                    for gi, (a, b2) in enumerate(out_groups):
                        if b2 == g + 1:
                            lo = a * ps_ch * B
                            hi = b2 * ps_ch * B
                            # Outs ride the HWDGE rings, issued strictly
                            # AFTER every W dma_start on that engine: their
                            # descriptors queue behind all W in ring FIFO,
                            # so the input stream runs with zero SWDGE
                            # contention (input crawled at <100 GB/s during
                            # SWDGE out bursts), and the cast-waits cannot
                            # delay any W descriptor generation.
                            oeng = nc.sync if gi % 2 == 0 else nc.scalar
                            oeng.dma_start(
                                outr[:, lo:hi], OBALL[:, lo:hi]
                            )
            else:
                nch = tile_ch * wcols_per_ch
                for t in range(n_tiles):
                    WT = wpa.tile([IN_D, nch], w_dt)
                    # Alternate the two HWDGE rings so W transfers overlap.
                    weng = nc.sync if t % 2 == 0 else nc.scalar
                    weng.dma_start(WT[:], wr[:, t * nch:(t + 1) * nch])
                    for h in range(banks_per_tile):
                        g = t * banks_per_tile + h
                        PS = pp.tile([OUT_D, BANK_CH * B], F32)
                        if has_bias:
                            # Seed bank: PS[o, j*16+b] = bias[g*32+j, o].
                            nc.tensor.matmul(
                                PS[:],
                                CB[:, g * OUT_D:(g + 1) * OUT_D],
                                CB[:, BN_COLS:CB_COLS],
                                start=True,
                                stop=False,
                            )
                        for j in range(BANK_CH):
                            jt = h * BANK_CH + j
                            dl = g * BANK_CH + j
                            out_sl = PS[:, j * B:(j + 1) * B]
                            whi = WT[
                                :, jt * wcols_per_ch:jt * wcols_per_ch + OUT_D
                            ]
                            xhi = XC[:, dl * B:(dl + 1) * B]
                            nc.tensor.matmul(
                                out_sl,
                                whi,
                                xhi,
                                start=(not has_bias) and j == 0,
                                stop=(mode != "mix3") and (j == BANK_CH - 1),
                            )
                            if mode == "mix3":
                                wlo = WT[
                                    :,
                                    jt * wcols_per_ch + OUT_D:
                                    (jt + 1) * wcols_per_ch,
                                ]
                                xlo = XC[
                                    :, X_COLS + dl * B:X_COLS + (dl + 1) * B
                                ]
                                nc.tensor.matmul(
                                    out_sl, whi, xlo, start=False, stop=False
                                )
                                nc.tensor.matmul(
                                    out_sl,
                                    wlo,
                                    xhi,
                                    start=False,
                                    stop=(j == BANK_CH - 1),
                                )
                        OB = op.tile([OUT_D, BANK_CH * B], out_dt)
                        nc.vector.tensor_copy(OB[:], PS[:])
                        nc.sync.dma_start(
                            outr[:, g * BANK_CH * B:(g + 1) * BANK_CH * B],
                            OB[:],
                        )

    nc.finalize()
    return nc


def _pack_x(x, sl, perm=None):
    # [b, i, dslab] -> [i, dl*16+b], channels optionally permuted into
    # the per-ring-contiguous stream order.
    xs = x[:, :, sl].transpose(1, 2, 0)  # [i, dl, b]
    if perm is not None:
        xs = xs[:, perm, :]
    return np.ascontiguousarray(xs).reshape(IN_D, X_COLS)


def _pack_bias(b, sl, eh):
    bnr = np.ascontiguousarray(
        b[sl].reshape(N_BANKS, BANK_CH, OUT_D).transpose(1, 0, 2)
    ).reshape(BANK_CH, BN_COLS)
    cbv = np.zeros((BANK_CH, CB_COLS), dtype=ml_dtypes.bfloat16)
    cbv[:, :BN_COLS] = bnr.astype(ml_dtypes.bfloat16)
    cbv[:, BN_COLS:] = eh.astype(ml_dtypes.bfloat16)
    return cbv


def _prep_core_inputs(x, W, b, mode, has_bias):
    _, w_np, _, x_np = _DT[mode]
    if mode == "f8w":
        # Power-of-2 scale pair: exact in both dtypes, cancels in the
        # product, so no on-device descale is needed.
        x = x * (1.0 / W_SCALE)
        W = W * W_SCALE
    eh = np.repeat(np.eye(BANK_CH, dtype=np.float32), B, axis=1)
    if mode == "mix3":
        bf = ml_dtypes.bfloat16
        xh = x.astype(bf)
        xl = (x - xh.astype(np.float32)).astype(bf)
        Wh = W.astype(bf)
        Wl = (W - Wh.astype(np.float32)).astype(bf)
    in_maps = []
    for c in range(NCORES):
        sl = slice(c * D_C, (c + 1) * D_C)
        if mode == "mix3":
            xcv = np.concatenate(
                [_pack_x(xh.astype(np.float32), sl), _pack_x(xl.astype(np.float32), sl)],
                axis=1,
            ).astype(bf)
            wrv = np.ascontiguousarray(
                np.stack(
                    [Wh[sl].transpose(1, 0, 2), Wl[sl].transpose(1, 0, 2)],
                    axis=2,
                )
            ).reshape(IN_D, D_C * 2 * OUT_D)
            m = {"xc": xcv, "wr": wrv}
            if has_bias:
                m["cb"] = _pack_bias(b, sl, eh)
            in_maps.append(m)
            continue
        # The prefetch path (bf16/f8w, no bias) consumes per-ring-permuted
        # x; the streaming fallback uses plain channel order.
        use_perm = mode in ("bf16", "f8w") and not has_bias
        xr = _pack_x(x, sl, X_PERM if use_perm else None).astype(
            x_np, copy=False
        )
        wrv = (
            np.ascontiguousarray(W[sl].transpose(1, 0, 2))
            .reshape(IN_D, D_C * OUT_D)
            .astype(w_np, copy=False)
        )
        m = {"xc": xr, "wr": wrv}
        if has_bias:
            m["cb"] = _pack_bias(b, sl, eh)
        in_maps.append(m)
    return in_maps


def run(inputs, trace=False, mode=None):
    mode = mode or MM_DTYPE
    x = np.asarray(inputs["x"], dtype=np.float32)
    W = np.asarray(inputs["W"], dtype=np.float32)
    b = np.asarray(inputs["b"], dtype=np.float32)
    has_bias = bool(np.any(b))
    key = (mode, has_bias)
    if key not in _cached:
        _cached[key] = _build(mode, has_bias)
    in_maps = _prep_core_inputs(x, W, b, mode, has_bias)
    res = run_bass_kernel_spmd(
        _cached[key], in_maps, core_ids=list(range(NCORES)), trace=trace
    )
    out = np.empty((B, OUT_D, D_TOTAL), dtype=np.float32)
    for c in range(NCORES):
        sl = slice(c * D_C, (c + 1) * D_C)
        out[:, :, sl] = (
            np.asarray(res.results[c]["outr"])
            .astype(np.float32)
            .reshape(OUT_D, D_C, B)
            .transpose(2, 0, 1)
        )
    return out, res


def kernel(**inputs):
    out, _ = run(inputs)
    return out

